# revision 64
# baseline (speedup 1.0000x reference)
"""Trainium2 Bass kernel for nn_ChainLoss (LF-MMI style chain loss).

Algorithm (validated bit-exact vs reference in numpy):
  Log-domain HMM forward recursion done in exp-domain with periodic rescaling.
  One shared denominator graph (4000 states, 120k edges) + 32 per-utterance
  numerator graphs (200 states, 600 edges) are merged into one state table
  A[5120 rows x 32 utts] (fp32, stored 64-wide for 256B gather alignment):
    - shard c (rows 640c..640c+639): 512 den rows (500 used, global in-degree
      round-robin relabel) + 128 num rows (combined num state j lives at
      640*(j%8) + 512 + j//8; only cols = its utterance are nonzero).
  The 8 cores shard *states*: core c owns shard c and all in-edges targeting
  it, pre-sorted into a padded grid of 5 partition-tiles (4 den + 1 num
  sub-row tile; num state in-edges are split over 5 sub-rows, recombined with
  a small 0/1 matmul).

  Host->device transfer (the dominant cost through the axon tunnel) is
  minimized: x is int4-quantized (offset-8; dequant scale/bias folded into
  the Exp activation, ~1.7e-3 objf error vs 2e-2 tolerance), stored with
  utt-major row bytes so each 16-step chunk ships only its live utterance
  lanes (x_lengths trim, ~25% fewer bytes), round-robin sharded across
  cores; one device-side AllGather + 32 block copies assemble the full x
  table in HBM. All small side inputs ship as one packed fp16 tensor
  (converted on device), gather indices as one int16 tensor (16-partition
  base block, replicated to 128 partitions on device). The PJRT executor is
  built once and cached (run_bass_kernel_spmd re-traces its jit wrapper
  every call); gathers spread over 4 SWDGE queues.

  Per step:
    AllGather shards -> table T; dma_gather A[src] rows (256B descriptors) and
    packed-int4 x[t, pdf] rows (256B descriptors, 16 time-steps each, from a
    [32*3072, 256] time-chunked transpose of x; nibble-unpacked with
    shift/and every 4 steps); z = a_src * w * exp(x);
    free-axis reduce per tile -> new shard; per-utt length masking each step;
    rescale every 4 steps by column sums of a fixed table subset (tracked in
    log-space accumulators).
  Final: per-core partial sums of A_T * exp(final_lp) for den/num regions;
  host combines 8 partial vectors + log-scale accumulators into the scalar.
"""
import numpy as np

NCORES = 8
B = 32
T = 500
D = 3072
S_DEN = 4000
S_NUM = 200
DEN_ROWS = 512
SHARD = 640
NROWS = SHARD * NCORES      # 5120
NSUB = 5
XCH = 4                     # time steps per E-expansion group
XG = 16                     # time steps per X-gather descriptor (256B int4)
GCAP = 4096                 # max indices per dma_gather instruction
RS = 4                      # rescale every RS steps
NCHX = 32                   # padded x chunk count (32*16=512 >= T steps)
CPCX = NCHX // NCORES       # x time-chunks staged per core
S4 = 4.5 / 7.0              # int4 x quantization scale


# ---------------------------------------------------------------- host prep
def _preprocess(den_src, den_dst, den_pdf, den_logw, den_init, den_final,
                num_src, num_dst, num_pdf, num_logw, num_init, num_final,
                x_lengths):
    indeg = np.bincount(den_dst, minlength=S_DEN)
    rank_of_state = np.empty(S_DEN, np.int64)
    rank_of_state[np.argsort(-indeg, kind="stable")] = np.arange(S_DEN)
    core_of = rank_of_state % NCORES
    rowin = rank_of_state // NCORES
    rowof_den = core_of * SHARD + rowin
    rowof_num = (np.arange(S_NUM) % NCORES) * SHARD + DEN_ROWS + np.arange(S_NUM) // NCORES

    E = len(den_dst)
    core_e = core_of[den_dst]
    ri_e = rowin[den_dst]
    grp = core_e * DEN_ROWS + ri_e
    order = np.argsort(grp, kind="stable")
    grp_s = grp[order]
    first = np.r_[True, grp_s[1:] != grp_s[:-1]]
    start_pos = np.where(first, np.arange(E), 0)
    k_within = np.arange(E) - np.maximum.accumulate(start_pos)
    e_src = rowof_den[den_src[order]]
    e_pdf = den_pdf[order]
    e_w = np.exp(den_logw[order]).astype(np.float32)
    tile_s = ri_e[order] // 128
    part_s = ri_e[order] % 128
    core_s = core_e[order]

    per_core = [dict(aidx=[None] * 5, xidx=[None] * 5, w=[None] * 5)
                for _ in range(NCORES)]
    Kmax = [0] * 5
    raw = {}
    for c in range(NCORES):
        for j in range(4):
            sel = (core_s == c) & (tile_s == j)
            K = int(k_within[sel].max()) + 1 if sel.any() else 1
            Kmax[j] = max(Kmax[j], K)
            raw[(c, j)] = sel

    uu = np.repeat(np.arange(B), num_dst.shape[1])
    nd = num_dst.reshape(-1)
    ns = num_src.reshape(-1)
    npf = num_pdf.reshape(-1)
    nw = np.exp(num_logw.reshape(-1)).astype(np.float32)
    ncore = nd % NCORES
    jj = nd // NCORES
    grp = ncore * S_NUM + nd
    order_n = np.argsort(grp, kind="stable")
    grp_s = grp[order_n]
    first = np.r_[True, grp_s[1:] != grp_s[:-1]]
    start_pos = np.where(first, np.arange(len(nd)), 0)
    cum = np.arange(len(nd)) - np.maximum.accumulate(start_pos)
    part_n = jj[order_n] * NSUB + (cum % NSUB)
    slot_n = cum // NSUB
    for c in range(NCORES):
        sel = ncore[order_n] == c
        K = int(slot_n[sel].max()) + 1 if sel.any() else 1
        Kmax[4] = max(Kmax[4], K)
        raw[(c, 4)] = sel

    for c in range(NCORES):
        for j in range(4):
            K = Kmax[j]
            sel = raw[(c, j)]
            ai = np.zeros((128, K), np.int32)
            xi = np.zeros((128, K), np.int32)
            wt = np.zeros((128, K), np.float32)   # den w: same for all utts
            p, k = part_s[sel], k_within[sel]
            ai[p, k] = e_src[sel]
            xi[p, k] = e_pdf[sel]
            wt[p, k] = e_w[sel]
            pc = per_core[c]
            pc["aidx"][j] = ai; pc["xidx"][j] = xi; pc["w"][j] = wt
        K = Kmax[4]
        sel = raw[(c, 4)]
        ai = np.zeros((128, K), np.int32)
        xi = np.zeros((128, K), np.int32)
        wt = np.zeros((128, K, B), np.float32)
        p, k = part_n[sel], slot_n[sel]
        ai[p, k] = rowof_num[ns[order_n][sel]]
        xi[p, k] = npf[order_n][sel]
        wt[p, k, uu[order_n][sel]] = nw[order_n][sel]
        pc = per_core[c]
        pc["aidx"][4] = ai; pc["xidx"][4] = xi; pc["w"][4] = wt

    G = np.zeros((128, 128), np.float32)
    for q in range(S_NUM // NCORES):
        for m in range(NSUB):
            G[q * NSUB + m, q] = 1.0

    A0 = np.zeros((NROWS, B), np.float32)
    A0[rowof_den, :] = np.exp(den_init).astype(np.float32)[:, None]
    for u in range(B):
        A0[rowof_num, u] = np.exp(num_init[u]).astype(np.float32)
    F = np.zeros((NROWS, B), np.float32)
    F[rowof_den, :] = np.exp(den_final).astype(np.float32)[:, None]
    for u in range(B):
        F[rowof_num, u] = np.exp(num_final[u]).astype(np.float32)

    return per_core, Kmax, G, A0, F


def _wrap_idx(flat):
    # dma_gather index layout: flat index i -> [i%16, i//16]; shipped as the
    # 16-partition base block, replicated to 128 partitions on device.
    w = flat.reshape(-1, 16).T
    return np.ascontiguousarray(w.astype(np.int16))


# ------------------------------------------------------------- bass program
# variant: timing diagnostics only (results are wrong for variant != 0).
#   1 = per-step AllGather replaced by a local HBM copy
#   2 = variant 1 + all per-step dma_gathers skipped
# xplan: (SHB, O_ch, Pch) live-lane shipping plan — per-core padded shard
#   bytes, per-chunk byte offset in the AllGathered blob, per-chunk live
#   utterance-lane prefix length.
def _build(Kmax, n_steps, variant=0, xplan=None):
    import concourse.bass as bass
    import concourse.tile as tile
    from concourse import bacc, mybir

    f32 = mybir.dt.float32
    KTOT = sum(Kmax)
    NIDX = 128 * KTOT
    offs = np.cumsum([0] + Kmax).tolist()

    nc = bacc.Bacc("TRN2", target_bir_lowering=False, debug=False,
                   num_devices=NCORES, num_swdge_queues=4)
    core_ids = list(range(NCORES))
    SP = (variant == 5)        # timing experiment: single-packet gathers
    do_cc = variant in (0, 2, 5)       # real per-step AllGather
    do_gather = variant in (0, 1, 5)   # real per-step dma_gathers
    qn = [0]                # round-robin SWDGE queue assignment for gathers

    def next_q():
        qn[0] = (qn[0] + 1) % 4
        return qn[0]

    u8 = mybir.dt.uint8
    KD = offs[4]            # total den K slots
    KN = Kmax[4]            # num-tile K slots
    XROW = XG * B // 2      # packed bytes per x row (2 int4 steps/byte)

    # x staged packed int4 (scale S4; hi nibble = steps 0-7 of the chunk, lo
    # nibble = steps 8-15), time-chunk-sharded (core c holds XG-step chunks
    # CPCX*c..); one device-side AllGather assembles the full table in HBM.
    # (Collectives cannot read IO tensors, so bounce through an internal one.)
    # Rows are utt-major: byte u*8+s holds step 16ch+s (hi nibble) /
    # 16ch+8+s (lo) of utt u. Only the live utt-lane prefix of each chunk
    # ships (per xplan); chunks are round-robin balanced across cores,
    # AllGathered as a blob, then copied into the gather table (dead lanes
    # stay garbage -- harmless, they are masked by the length blend).
    SHB, O_ch, Pch = xplan
    xt8s_in = nc.dram_tensor("xt8s", [SHB // 256, 256], u8, kind="ExternalInput").ap()
    xt8loc = nc.dram_tensor("xt8loc", [SHB // 256, 256], u8).ap()
    xtblob = nc.dram_tensor("xtblob", [NCORES * SHB // 256, 256], u8,
                            addr_space="Shared").ap()
    xt8 = nc.dram_tensor("xt8g", [NCHX * D, XROW], u8).ap()
    # all small side inputs packed into one fp16 tensor (fewer PJRT
    # transfers, half the bytes); converted to f32 on device once:
    # wden | wnum | gmat | fshard | init(packed, alpha cols only) | len
    f16 = mybir.dt.float16
    o_wd = 0
    o_wn = o_wd + KD           # num-tile utt ids [128,KN], then weights
    o_wv = o_wn + KN
    o_io = o_wv + KN           # iota row 0..B-1 (all partitions)
    o_gm = o_io + B
    o_fs = o_gm + 128          # 4 den cols (utt-constant) + B num cols
    o_ip = o_fs + 4 + B
    o_ln = o_ip + 4 + B
    SC = o_ln + 64
    side_in = nc.dram_tensor("side", [128, SC], f16, kind="ExternalInput").ap()
    NC16 = NIDX // 16
    sidx_in = nc.dram_tensor("sidx", [16, 2 * NC16], mybir.dt.int16, kind="ExternalInput").ap()
    out_t = nc.dram_tensor("out", [4, B], f32, kind="ExternalOutput").ap()

    shard64 = nc.dram_tensor("shard64", [SHARD, 64], f32).ap()
    TT = [nc.dram_tensor(f"table{i}", [NROWS, 64], f32, addr_space="Shared").ap()
          for i in range(2)]

    with tile.TileContext(nc) as tc:
        with tc.tile_pool(name="main", bufs=1) as pool, \
             tc.tile_pool(name="psum", bufs=1, space="PSUM") as psum:

            # assemble the x blob on device (D2D), then expand live-lane
            # chunk blocks into the fixed-stride gather table
            nc.scalar.dma_start(out=xt8loc[:], in_=xt8s_in[:])
            nc.gpsimd.collective_compute(
                "AllGather", mybir.AluOpType.bypass,
                replica_groups=[core_ids],
                ins=[xt8loc[:]], outs=[xtblob[:]])
            for ch in range(NCHX):
                P = Pch[ch]
                if P == 0:
                    continue
                nc.scalar.dma_start(
                    out=bass.AP(xt8.tensor, ch * D * XROW, [(XROW, D), (1, P * 8)]),
                    in_=bass.AP(xtblob.tensor, O_ch[ch], [(P * 8, D), (1, P * 8)]))

            # indices shipped as the 16-partition base block; replicate to 128
            aidx_t = pool.tile([128, NC16], mybir.dt.int16)
            xidx_t = pool.tile([128, NC16], mybir.dt.int16)
            for g in range(8):
                nc.sync.dma_start(out=aidx_t[16 * g:16 * (g + 1), :],
                                  in_=sidx_in[:, 0:NC16])
                nc.sync.dma_start(out=xidx_t[16 * g:16 * (g + 1), :],
                                  in_=sidx_in[:, NC16:2 * NC16])
            sstage = pool.tile([128, SC], f16)
            nc.sync.dma_start(out=sstage[:], in_=side_in[:])
            wden = pool.tile([128, KD], f32)
            nc.vector.tensor_copy(out=wden[:], in_=sstage[:, o_wd:o_wd + KD])
            # expand one-hot num weights: wnum[p,k,u] = wval*(u == wid)
            wid = pool.tile([128, KN], f32)
            nc.vector.tensor_copy(out=wid[:], in_=sstage[:, o_wn:o_wn + KN])
            wval = pool.tile([128, KN], f32)
            nc.vector.tensor_copy(out=wval[:], in_=sstage[:, o_wv:o_wv + KN])
            iota = pool.tile([128, B], f32)
            nc.vector.tensor_copy(out=iota[:], in_=sstage[:, o_io:o_io + B])
            wnum = pool.tile([128, KN, B], f32)
            nc.vector.tensor_tensor(
                out=wnum[:],
                in0=wid[:].unsqueeze(2).to_broadcast([128, KN, B]),
                in1=iota[:].unsqueeze(1).to_broadcast([128, KN, B]),
                op=mybir.AluOpType.is_equal)
            nc.vector.tensor_tensor(
                out=wnum[:], in0=wnum[:],
                in1=wval[:].unsqueeze(2).to_broadcast([128, KN, B]),
                op=mybir.AluOpType.mult)
            gmat = pool.tile([128, 128], f32)
            nc.vector.tensor_copy(out=gmat[:], in_=sstage[:, o_gm:o_gm + 128])
            fshard = pool.tile([128, 5, B], f32)
            nc.vector.tensor_copy(
                out=fshard[:, 0:4, :],
                in_=sstage[:, o_fs:o_fs + 4].unsqueeze(2).to_broadcast([128, 4, B]))
            nc.vector.tensor_copy(
                out=fshard[:, 4:5, :],
                in_=sstage[:, o_fs + 4:o_fs + 4 + B].unsqueeze(1))
            len64 = pool.tile([1, 64], f32)
            nc.vector.tensor_copy(out=len64[:], in_=sstage[0:1, o_ln:o_ln + 64])

            ones128 = pool.tile([128, 1], f32)
            nc.vector.memset(ones128[:], 1.0)
            ones1r = pool.tile([1, 128], f32)
            nc.vector.memset(ones1r[:], 1.0)
            logs64 = pool.tile([1, 64], f32)
            nc.vector.memset(logs64[:], 0.0)

            # shard ping-pong tiles ([p, tile, utt]); shard_t[t%2] = alpha_t
            shard_t = [pool.tile([128, 5, B], f32, name=f"shard{i}") for i in range(2)]
            iden = sstage[:, o_ip:o_ip + 4].unsqueeze(2).to_broadcast([128, 4, B])
            inum = sstage[:, o_ip + 4:o_ip + 4 + B].unsqueeze(1)
            nc.vector.tensor_copy(out=shard_t[0][:, 0:4, :], in_=iden)
            nc.vector.tensor_copy(out=shard_t[0][:, 4:5, :], in_=inum)
            # shard64 internal := initial shard, alpha in cols 0:B, zeros after
            s64init = pool.tile([128, 5, 64], f32)
            nc.vector.memset(s64init[:], 0.0)
            nc.vector.tensor_copy(out=s64init[:, 0:4, 0:B], in_=iden)
            nc.vector.tensor_copy(out=s64init[:, 4:5, 0:B], in_=inum)
            for jj in range(5):
                nc.scalar.dma_start(out=shard64[jj * 128:(jj + 1) * 128, :],
                                    in_=s64init[:, jj, :])

            ga_t = [pool.tile([128, Kmax[j], 64], f32, name=f"ga{j}")
                    for j in range(5)]
            gx_t = [pool.tile([128, Kmax[j], XCH * B], f32, name=f"gx{j}")
                    for j in range(5)]
            gxq_t = [pool.tile([128, Kmax[j], XROW], u8, name=f"gxq{j}")
                     for j in range(5)]
            if variant >= 2:
                # timing diagnostics: no gathers in-loop, so initialize once
                for j in range(5):
                    nc.vector.memset(ga_t[j][:], 0.5)
                    nc.vector.memset(gxq_t[j][:], 0x88)
            KMX = max(Kmax)
            nib_t = [pool.tile([128, KMX, XCH * B], u8, name=f"nib{i}")
                     for i in range(2)]
            bias_t = pool.tile([128, 1], f32)
            nc.vector.memset(bias_t[:], -8.0 * S4)
            srows = pool.tile([128, B], f32)
            numreg = pool.tile([25, 8 * B], f32)
            s64 = pool.tile([1, 64], f32)
            inv64 = pool.tile([1, 64], f32)
            ln64 = pool.tile([1, 64], f32)
            m64 = pool.tile([1, 64], f32)
            ccat = pool.tile([1, 128], f32)
            cb = pool.tile([128, 128], f32)
            tmp5 = pool.tile([128, 5, B], f32)
            tmp64 = pool.tile([1, 64], f32)

            for t in range(n_steps):
                T_dst = TT[t % 2]
                a_old = shard_t[t % 2]
                a_new = shard_t[(t + 1) % 2]
                rescale = (t % RS == RS - 1)

                # 1. exchange shards -> full table for this step
                if do_cc:
                    nc.gpsimd.collective_compute(
                        "AllGather", mybir.AluOpType.bypass,
                        replica_groups=[core_ids],
                        ins=[shard64[:]], outs=[T_dst[:]])
                else:
                    nc.scalar.dma_start(out=T_dst[0:SHARD, :], in_=shard64[:])

                # 2. gathers, split per grid tile (and per <=GCAP chunk)
                # so tile j's compute overlaps later tiles' gathers
                q = t % XCH
                chx = t // XG
                grp = (t % XG) // XCH       # 4-step group within the chunk
                for j in range(5):
                    base = offs[j] * 128
                    nj = Kmax[j] * 128
                    if t % XG == 0 and do_gather:
                        # packed int4 x rows for the next 16 steps
                        for o in range(0, nj, GCAP):
                            n = min(GCAP, nj - o)
                            go, gn = (base + o), n
                            nc.gpsimd.dma_gather(
                                gxq_t[j][:, o // 128:(o + n) // 128, :],
                                xt8[chx * D:(chx + 1) * D, :],
                                xidx_t[:, go // 16:(go + gn) // 16], n, n,
                                XROW, single_packet=SP, queue_num=next_q())
                    if q == 0:
                        # unpack nibble (hi: steps 0-7, lo: 8-15) from the
                        # utt-major rows into step-major [s*32+u] order, then
                        # E' = exp(S4*(v-8)) * w for 4 steps
                        nib = nib_t[j % 2][:, 0:Kmax[j], :]
                        nib4 = nib.rearrange("p k (s u) -> p k s u", s=XCH)
                        src4 = gxq_t[j][:] \
                            .rearrange("p k (u s) -> p k u s", u=B) \
                            [:, :, :, 4 * (grp % 2):4 * (grp % 2) + 4] \
                            .transpose([0, 1, 3, 2])
                        if grp < 2:
                            nc.vector.tensor_scalar(
                                out=nib4, in0=src4, scalar1=4, scalar2=None,
                                op0=mybir.AluOpType.logical_shift_right)
                        else:
                            nc.vector.tensor_scalar(
                                out=nib4, in0=src4, scalar1=15, scalar2=None,
                                op0=mybir.AluOpType.bitwise_and)
                        nc.scalar.activation(
                            out=gx_t[j][:], in_=nib,
                            func=mybir.ActivationFunctionType.Exp,
                            scale=S4, bias=bias_t[:])
                        if j < 4:
                            wb = wden[:, offs[j]:offs[j + 1]].unsqueeze(2) \
                                .unsqueeze(3).to_broadcast([128, Kmax[j], XCH, B])
                        else:
                            wb = wnum[:].unsqueeze(2) \
                                .to_broadcast([128, Kmax[j], XCH, B])
                        nc.vector.tensor_tensor(
                            out=gx_t[j][:].rearrange("p k (s b) -> p k s b", s=XCH),
                            in0=gx_t[j][:].rearrange("p k (s b) -> p k s b", s=XCH),
                            in1=wb, op=mybir.AluOpType.mult)
                    if do_gather:
                        for o in range(0, nj, GCAP):
                            n = min(GCAP, nj - o)
                            go, gn = (base + o), n
                            nc.gpsimd.dma_gather(
                                ga_t[j][:, o // 128:(o + n) // 128, :], T_dst[:],
                                aidx_t[:, go // 16:(go + gn) // 16], n, n, 64,
                                single_packet=SP, queue_num=next_q())

                # 3+4. per tile: z = a_src * (w*exp(x)), reduce over slots
                for j in range(5):
                    gav = ga_t[j][:, :, 0:B]
                    nc.vector.tensor_tensor(
                        out=gav, in0=gav,
                        in1=gx_t[j][:, :, q * B:(q + 1) * B],
                        op=mybir.AluOpType.mult)
                    nc.vector.tensor_reduce(
                        out=a_new[:, j, :],
                        in_=gav.transpose([0, 2, 1]),
                        axis=mybir.AxisListType.X,
                        op=mybir.AluOpType.add)

                # 5. num sub-row combine
                pnum = psum.tile([128, B], f32, space="PSUM")
                nc.tensor.matmul(out=pnum[:], lhsT=gmat[:], rhs=a_new[:, 4, :],
                                 start=True, stop=True)
                nc.vector.tensor_copy(out=a_new[:, 4, :], in_=pnum[:])

                # 6. masks + (periodic) scales
                nc.vector.tensor_scalar(
                    out=m64[:], in0=len64[:], scalar1=float(t), scalar2=None,
                    op0=mybir.AluOpType.is_gt)
                if rescale:
                    nc.scalar.dma_start(out=srows[:], in_=T_dst[0:128, 0:B])
                    nreg_view = bass.AP(T_dst.tensor, DEN_ROWS * 64,
                                        [(64, 25), (SHARD * 64, 8), (1, B)])
                    nc.scalar.dma_start(out=numreg[:], in_=nreg_view)
                    ps1 = psum.tile([1, B], f32, space="PSUM")
                    nc.tensor.matmul(out=ps1[:], lhsT=ones128[:], rhs=srows[:],
                                     start=True, stop=True)
                    nc.vector.tensor_copy(out=s64[0:1, 0:B], in_=ps1[:])
                    ps2 = psum.tile([1, 8 * B], f32, space="PSUM")
                    nc.tensor.matmul(out=ps2[:], lhsT=ones128[0:25, :],
                                     rhs=numreg[:], start=True, stop=True)
                    nc.vector.tensor_reduce(
                        out=s64[0:1, B:2 * B],
                        in_=ps2[:].rearrange("o (c b) -> o c b", c=8).transpose([0, 2, 1]),
                        axis=mybir.AxisListType.X, op=mybir.AluOpType.add)
                    nc.vector.tensor_scalar(
                        out=s64[:], in0=s64[:], scalar1=1e-30, scalar2=None,
                        op0=mybir.AluOpType.max)
                    nc.vector.reciprocal(out=inv64[:], in_=s64[:])
                    nc.scalar.activation(out=ln64[:], in_=s64[:],
                                         func=mybir.ActivationFunctionType.Ln)
                    nc.vector.tensor_tensor(out=tmp64[:], in0=m64[:], in1=ln64[:],
                                            op=mybir.AluOpType.mult)
                    nc.vector.tensor_tensor(out=logs64[:], in0=logs64[:],
                                            in1=tmp64[:], op=mybir.AluOpType.add)
                    nc.vector.tensor_tensor(out=ccat[0:1, 0:64], in0=m64[:],
                                            in1=inv64[:], op=mybir.AluOpType.mult)
                else:
                    nc.vector.tensor_copy(out=ccat[0:1, 0:64], in_=m64[:])
                # C2 = 1 - m  (both halves share m; write den/num halves)
                nc.vector.tensor_scalar(
                    out=tmp64[:], in0=m64[:], scalar1=-1.0, scalar2=1.0,
                    op0=mybir.AluOpType.mult, op1=mybir.AluOpType.add)
                nc.vector.tensor_copy(out=ccat[0:1, 64:128], in_=tmp64[:])

                # broadcast [1,128] -> [128,128]
                pbc = psum.tile([128, 128], f32, space="PSUM")
                nc.tensor.matmul(out=pbc[:], lhsT=ones1r[:],
                                 rhs=ccat[:], start=True, stop=True)
                nc.vector.tensor_copy(out=cb[:], in_=pbc[:])

                # 7. a_new = C1*a_new + C2*a_old
                c1_den = cb[:, 0:B].unsqueeze(1).to_broadcast([128, 4, B])
                c1_num = cb[:, B:2 * B].unsqueeze(1).to_broadcast([128, 1, B])
                c2_den = cb[:, 2 * B:3 * B].unsqueeze(1).to_broadcast([128, 4, B])
                c2_num = cb[:, 3 * B:4 * B].unsqueeze(1).to_broadcast([128, 1, B])
                nc.vector.tensor_tensor(out=a_new[:, 0:4, :], in0=a_new[:, 0:4, :],
                                        in1=c1_den, op=mybir.AluOpType.mult)
                nc.vector.tensor_tensor(out=a_new[:, 4:5, :], in0=a_new[:, 4:5, :],
                                        in1=c1_num, op=mybir.AluOpType.mult)
                nc.vector.tensor_tensor(out=tmp5[:, 0:4, :], in0=a_old[:, 0:4, :],
                                        in1=c2_den, op=mybir.AluOpType.mult)
                nc.vector.tensor_tensor(out=tmp5[:, 4:5, :], in0=a_old[:, 4:5, :],
                                        in1=c2_num, op=mybir.AluOpType.mult)
                nc.vector.tensor_tensor(out=a_new[:], in0=a_new[:], in1=tmp5[:],
                                        op=mybir.AluOpType.add)

                # 8. write shard for next exchange
                sh_view = bass.AP(shard64.tensor, 0, [(64, 128), (128 * 64, 5), (1, B)])
                nc.sync.dma_start(out=sh_view, in_=a_new[:])

            # ---- final partials ----
            a_fin = shard_t[n_steps % 2]
            nc.vector.tensor_tensor(out=a_fin[:], in0=a_fin[:], in1=fshard[:],
                                    op=mybir.AluOpType.mult)
            pd = psum.tile([1, 4 * B], f32, space="PSUM")
            nc.tensor.matmul(out=pd[:], lhsT=ones128[:],
                             rhs=a_fin[:, 0:4, :], start=True, stop=True)
            den_part = pool.tile([1, B], f32)
            nc.vector.tensor_reduce(
                out=den_part[:],
                in_=pd[:].rearrange("o (j b) -> o j b", j=4).transpose([0, 2, 1]),
                axis=mybir.AxisListType.X, op=mybir.AluOpType.add)
            pn = psum.tile([1, B], f32, space="PSUM")
            nc.tensor.matmul(out=pn[:], lhsT=ones128[:], rhs=a_fin[:, 4, :],
                             start=True, stop=True)
            num_part = pool.tile([1, B], f32)
            nc.vector.tensor_copy(out=num_part[:], in_=pn[:])

            nc.sync.dma_start(out=out_t[0:1, :], in_=den_part[:])
            nc.sync.dma_start(out=out_t[1:2, :], in_=num_part[:])
            nc.sync.dma_start(out=out_t[2:3, :], in_=logs64[0:1, 0:B])
            nc.sync.dma_start(out=out_t[3:4, :], in_=logs64[0:1, B:2 * B])

    nc.compile()
    return nc


_CACHE = {}


def _get_program(Kmax, n_steps, variant=0, xplan=None):
    key = (tuple(Kmax), n_steps, variant, xplan)
    if key not in _CACHE:
        _CACHE[key] = _build(Kmax, n_steps, variant, xplan)
    return _CACHE[key]


_EXEC_CACHE = {}


def _get_executor(nc):
    """Cached equivalent of bass2jax.run_bass_via_pjrt's inner jit call.

    run_bass_via_pjrt rebuilds jax.jit(shard_map(_body)) on every
    invocation, which re-traces and re-lowers (~1.4s/call of pure host
    overhead). Build the identical callable once per program and reuse it;
    the NEFF, transfers, and device execution are unchanged.
    """
    if id(nc) in _EXEC_CACHE:
        return _EXEC_CACHE[id(nc)]
    import jax
    from jax.sharding import Mesh, PartitionSpec
    from jax.experimental.shard_map import shard_map
    from concourse import mybir
    from concourse.bass2jax import (_bass_exec_p, install_neuronx_cc_hook,
                                    partition_id_tensor)

    install_neuronx_cc_hook()
    partition_name = (nc.partition_id_tensor.name
                      if nc.partition_id_tensor else None)
    in_names, out_names, out_avals, zero_outs = [], [], [], []
    for alloc in nc.m.functions[0].allocations:
        if not isinstance(alloc, mybir.MemoryLocationSet):
            continue
        name = alloc.memorylocations[0].name
        if alloc.kind == "ExternalInput":
            if name != partition_name:
                in_names.append(name)
        elif alloc.kind == "ExternalOutput":
            shape = tuple(alloc.tensor_shape)
            dtype = mybir.dt.np(alloc.dtype)
            out_avals.append(jax.core.ShapedArray(shape, dtype))
            zero_outs.append(np.zeros(shape, dtype))
            out_names.append(name)
    n_params = len(in_names)
    n_outs = len(out_avals)
    in_names_all = in_names + out_names
    if partition_name is not None:
        in_names_all.append(partition_name)

    def _body(*args):
        operands = list(args)
        if partition_name is not None:
            operands.append(partition_id_tensor())
        outs = _bass_exec_p.bind(
            *operands, out_avals=tuple(out_avals),
            in_names=tuple(in_names_all), out_names=tuple(out_names),
            lowering_input_output_aliases=(), sim_require_finite=True,
            sim_require_nnan=True, nc=nc)
        return tuple(outs)

    devices = jax.devices()[:NCORES]
    mesh = Mesh(np.asarray(devices), ("core",))
    in_specs = (PartitionSpec("core"),) * (n_params + n_outs)
    out_specs = (PartitionSpec("core"),) * n_outs
    donate = tuple(range(n_params, n_params + n_outs))
    sharded = jax.jit(shard_map(_body, mesh=mesh, in_specs=in_specs,
                                out_specs=out_specs, check_rep=False),
                      donate_argnums=donate, keep_unused=True)
    entry = (sharded, in_names, out_names, out_avals, zero_outs, mesh, devices)
    _EXEC_CACHE[id(nc)] = entry
    return entry


def _run_cached(nc, in_maps):
    """Execute via the cached jit callable; mirrors run_bass_via_pjrt.

    Inputs are staged with per-device device_puts issued from a thread pool
    (the axon tunnel gains ~25% from concurrent streams), then assembled
    into global sharded arrays without any host-side concatenation.
    """
    import jax
    from concurrent.futures import ThreadPoolExecutor
    from jax.sharding import NamedSharding, PartitionSpec

    sharded, in_names, out_names, out_avals, zero_outs, mesh, devices = \
        _get_executor(nc)
    sh = NamedSharding(mesh, PartitionSpec("core"))

    jobs = [(nm, c) for nm in in_names for c in range(NCORES)]

    def put(job):
        nm, c = job
        return jax.device_put(np.asarray(in_maps[c][nm]), devices[c])

    with ThreadPoolExecutor(8) as ex:
        futs = list(ex.map(put, jobs))
    dev_in = []
    for i, nm in enumerate(in_names):
        shards = futs[i * NCORES:(i + 1) * NCORES]
        shape0 = shards[0].shape
        gshape = (NCORES * shape0[0], *shape0[1:])
        dev_in.append(jax.make_array_from_single_device_arrays(
            gshape, sh, shards))
    concat_zeros = [np.zeros((NCORES * z.shape[0], *z.shape[1:]), z.dtype)
                    for z in zero_outs]
    out_arrs = sharded(*dev_in, *concat_zeros)
    return [
        {nm: np.asarray(out_arrs[i]).reshape(NCORES, *out_avals[i].shape)[c]
         for i, nm in enumerate(out_names)}
        for c in range(NCORES)]


LAST_EXEC_NS = None
LAST_RUN_S = None


def _prepare(x, x_lengths, den_src, den_dst, den_pdf, den_logw, den_init,
             den_final, num_src, num_dst, num_pdf, num_logw, num_init,
             num_final, n_steps=T, _variant=0):
    x = np.asarray(x, np.float32)
    x_lengths_np = np.asarray(x_lengths)
    args = [np.asarray(a) for a in (den_src, den_dst, den_pdf, den_logw,
                                    den_init, den_final, num_src, num_dst,
                                    num_pdf, num_logw, num_init, num_final)]
    per_core, Kmax, G, A0, F = _preprocess(*args, x_lengths_np)
    KTOT = sum(Kmax)

    # x -> int4 quantized (offset-8, scale S4), packed 2 steps/byte: row
    # (ch*D + p), byte s*B+u holds step 16ch+s in the hi nibble and step
    # 16ch+8+s in the lo nibble, for all utts. Staged time-chunk-sharded
    # (CPCX chunks per core, zero-padded to NCHX chunks); one device-side
    # AllGather assembles the full table.
    q4 = (np.clip(np.rint(x * (1.0 / S4)), -8, 7) + 8).astype(np.uint8)
    xt = np.zeros((NCHX * XG, D, B), np.uint8)
    xt[:T] = q4.transpose(1, 2, 0)               # [T, D, B]
    t16 = xt.reshape(NCHX, 2, XG // 2, D, B)     # [ch, half, s, D, B]
    xutt = np.ascontiguousarray(
        ((t16[:, 0] << 4) | t16[:, 1])           # [NCHX, s(8), D, B]
        .transpose(0, 2, 3, 1))                  # [NCHX, D, B, 8] utt-major
    # live-lane shipping plan: per chunk, only the utt-lane prefix that is
    # still inside some utterance's length ships; chunks are round-robin
    # assigned to cores to balance bytes, shards padded to a common size.
    lens = x_lengths_np.astype(np.int64)
    Pch, sizes = [], []
    for ch in range(NCHX):
        alive = np.nonzero(lens > XG * ch)[0]
        P = int(alive.max()) + 1 if alive.size else 0
        Pch.append(P)
        sizes.append(D * 8 * P)
    order = [[] for _ in range(NCORES)]
    loads = [0] * NCORES
    for ch in sorted(range(NCHX), key=lambda k: -sizes[k]):
        c = min(range(NCORES), key=lambda k: loads[k])
        order[c].append(ch)
        loads[c] += sizes[ch]
    core_bytes = loads
    SHB = 256 * ((max(core_bytes) + 255) // 256)
    O_ch = [0] * NCHX
    xt8_shards = []
    for c in range(NCORES):
        blob = np.zeros(SHB, np.uint8)
        off = 0
        for ch in order[c]:
            n = sizes[ch]
            O_ch[ch] = c * SHB + off
            if n:
                blob[off:off + n] = xutt[ch, :, :Pch[ch], :].reshape(-1)
            off += n
        xt8_shards.append(blob.reshape(SHB // 256, 256))
    xplan = (SHB, tuple(O_ch), tuple(Pch))

    len64 = np.zeros(64, np.float32)
    len64[0:B] = x_lengths_np.astype(np.float32)
    len64[B:2 * B] = x_lengths_np.astype(np.float32)

    # packed side-tensor layout (must match _build):
    KD = sum(Kmax[0:4])
    KN = Kmax[4]
    o_wd, o_wn = 0, KD
    o_wv = o_wn + KN
    o_io = o_wv + KN
    o_gm = o_io + B
    o_fs = o_gm + 128          # 4 den cols (utt-constant) + B num cols
    o_ip = o_fs + 4 + B
    o_ln = o_ip + 4 + B
    SC = o_ln + 64

    in_maps = []
    for c in range(NCORES):
        pc = per_core[c]
        aflat = np.concatenate([pc["aidx"][j].T.reshape(-1) for j in range(5)])
        xflat = np.concatenate([pc["xidx"][j].T.reshape(-1) for j in range(5)])
        # index order: i = (off_j + k)*128 + p  -> per tile k-major, partition
        # fastest; aidx[j].T is [K, 128] -> reshape(-1) gives exactly that.
        fsh = F[c * SHARD:(c + 1) * SHARD, :]     # [640, B]

        side = np.zeros((128, SC), np.float16)
        side[:, o_wd:o_wn] = np.concatenate([pc["w"][j] for j in range(4)], axis=1)
        w4 = pc["w"][4]                       # [128, KN, B] one-hot per slot
        side[:, o_wn:o_wv] = np.where(w4.any(axis=2), w4.argmax(axis=2), B + 7)
        side[:, o_wv:o_io] = w4.max(axis=2)
        side[:, o_io:o_gm] = np.arange(B, dtype=np.float16)[None, :]
        side[:, o_gm:o_fs] = G
        fsh5 = fsh.reshape(5, 128, B)
        a05 = A0[c * SHARD:(c + 1) * SHARD, :].reshape(5, 128, B)
        for j in range(4):
            side[:, o_fs + j] = fsh5[j, :, 0]
            side[:, o_ip + j] = a05[j, :, 0]
        side[:, o_fs + 4:o_fs + 4 + B] = fsh5[4]
        side[:, o_ip + 4:o_ip + 4 + B] = a05[4]
        side[0, o_ln:SC] = len64
        sidx = np.concatenate([_wrap_idx(aflat.astype(np.int16)),
                               _wrap_idx(xflat.astype(np.int16))], axis=1)
        in_maps.append({
            "xt8s": xt8_shards[c],
            "side": side,
            "sidx": np.ascontiguousarray(sidx),
        })

    nc = _get_program(Kmax, n_steps, _variant, xplan)
    return nc, in_maps, x_lengths_np


def kernel(x, x_lengths, den_src, den_dst, den_pdf, den_logw, den_init, den_final,
           num_src, num_dst, num_pdf, num_logw, num_init, num_final,
           n_steps=T, _want_results=False, _trace=False, _variant=0):
    global LAST_EXEC_NS, LAST_RUN_S
    import time as _time

    nc, in_maps, x_lengths_np = _prepare(
        x, x_lengths, den_src, den_dst, den_pdf, den_logw, den_init,
        den_final, num_src, num_dst, num_pdf, num_logw, num_init, num_final,
        n_steps=n_steps, _variant=_variant)
    _t0 = _time.time()
    try:
        results = _run_cached(nc, in_maps)
    except Exception:
        # fall back to the stock SPMD runner
        from concourse.bass_utils import run_bass_kernel_spmd
        res = run_bass_kernel_spmd(nc, in_maps, core_ids=list(range(NCORES)))
        results = res.results
    LAST_RUN_S = _time.time() - _t0
    outs = [results[c]["out"] for c in range(NCORES)]
    if _want_results:
        return outs, results

    den_tot = np.sum([o[0] for o in outs], axis=0)
    num_tot = np.sum([o[1] for o in outs], axis=0)
    logs_den = outs[0][2]
    logs_num = outs[0][3]
    den_ll = np.log(np.maximum(den_tot, 1e-300)) + logs_den
    num_ll = np.log(np.maximum(num_tot, 1e-300)) + logs_num
    objf = -(num_ll.sum() - den_ll.sum()) / x_lengths_np.sum()
    return np.float32(objf)



# revision 68
# speedup vs baseline: 1.0827x; 1.0827x over previous
"""Trainium2 Bass kernel for nn_ChainLoss (LF-MMI style chain loss).

Algorithm (validated bit-exact vs reference in numpy):
  Log-domain HMM forward recursion done in exp-domain with periodic rescaling.
  One shared denominator graph (4000 states, 120k edges) + 32 per-utterance
  numerator graphs (200 states, 600 edges) are merged into one state table
  A[5120 rows x 32 utts] (fp32, stored 64-wide for 256B gather alignment):
    - shard c (rows 640c..640c+639): 512 den rows (500 used, global in-degree
      round-robin relabel) + 128 num rows (combined num state j lives at
      640*(j%8) + 512 + j//8; only cols = its utterance are nonzero).
  The 8 cores shard *states*: core c owns shard c and all in-edges targeting
  it, pre-sorted into a padded grid of 5 partition-tiles (4 den + 1 num
  sub-row tile; num state in-edges are split over 5 sub-rows, recombined with
  a small 0/1 matmul).

  Host->device transfer (the dominant cost through the axon tunnel) is
  minimized: x is int4-quantized (offset-8; dequant scale/bias folded into
  the Exp activation, ~1.7e-3 objf error vs 2e-2 tolerance), stored with
  utt-major row bytes so each 16-step chunk ships only its live utterance
  lanes (x_lengths trim, ~25% fewer bytes), round-robin sharded across
  cores; one device-side AllGather + 32 block copies assemble the full x
  table in HBM. All small side inputs ship as one packed fp16 tensor
  (converted on device), gather indices as one int16 tensor (16-partition
  base block, replicated to 128 partitions on device). The PJRT executor is
  built once and cached (run_bass_kernel_spmd re-traces its jit wrapper
  every call); gathers spread over 4 SWDGE queues.

  Per step:
    AllGather shards -> table T; dma_gather A[src] rows (256B descriptors) and
    packed-int4 x[t, pdf] rows (256B descriptors, 16 time-steps each, from a
    [32*3072, 256] time-chunked transpose of x; nibble-unpacked with
    shift/and every 4 steps); z = a_src * w * exp(x);
    free-axis reduce per tile -> new shard; per-utt length masking each step;
    rescale every 4 steps by column sums of a fixed table subset (tracked in
    log-space accumulators).
  Final: per-core partial sums of A_T * exp(final_lp) for den/num regions;
  host combines 8 partial vectors + log-scale accumulators into the scalar.
"""
import numpy as np

NCORES = 8
B = 32
T = 500
D = 3072
S_DEN = 4000
S_NUM = 200
DEN_ROWS = 512
SHARD = 640
NROWS = SHARD * NCORES      # 5120
NSUB = 5
XCH = 4                     # time steps per E-expansion group
XG = 16                     # time steps per X-gather descriptor (256B int4)
GCAP = 2048                 # max indices per dma_gather instruction
RS = 4                      # rescale every RS steps
NCHX = 32                   # padded x chunk count (32*16=512 >= T steps)
CPCX = NCHX // NCORES       # x time-chunks staged per core
S4 = 4.5 / 7.0              # int4 x quantization scale


# ---------------------------------------------------------------- host prep
def _preprocess(den_src, den_dst, den_pdf, den_logw, den_init, den_final,
                num_src, num_dst, num_pdf, num_logw, num_init, num_final,
                x_lengths):
    indeg = np.bincount(den_dst, minlength=S_DEN)
    rank_of_state = np.empty(S_DEN, np.int64)
    rank_of_state[np.argsort(-indeg, kind="stable")] = np.arange(S_DEN)
    core_of = rank_of_state % NCORES
    rowin = rank_of_state // NCORES
    rowof_den = core_of * SHARD + rowin
    rowof_num = (np.arange(S_NUM) % NCORES) * SHARD + DEN_ROWS + np.arange(S_NUM) // NCORES

    E = len(den_dst)
    core_e = core_of[den_dst]
    ri_e = rowin[den_dst]
    grp = core_e * DEN_ROWS + ri_e
    order = np.argsort(grp, kind="stable")
    grp_s = grp[order]
    first = np.r_[True, grp_s[1:] != grp_s[:-1]]
    start_pos = np.where(first, np.arange(E), 0)
    k_within = np.arange(E) - np.maximum.accumulate(start_pos)
    e_src = rowof_den[den_src[order]]
    e_pdf = den_pdf[order]
    e_w = np.exp(den_logw[order]).astype(np.float32)
    tile_s = ri_e[order] // 128
    part_s = ri_e[order] % 128
    core_s = core_e[order]

    per_core = [dict(aidx=[None] * 5, xidx=[None] * 5, w=[None] * 5)
                for _ in range(NCORES)]
    Kmax = [0] * 5
    raw = {}
    for c in range(NCORES):
        for j in range(4):
            sel = (core_s == c) & (tile_s == j)
            K = int(k_within[sel].max()) + 1 if sel.any() else 1
            Kmax[j] = max(Kmax[j], K)
            raw[(c, j)] = sel

    uu = np.repeat(np.arange(B), num_dst.shape[1])
    nd = num_dst.reshape(-1)
    ns = num_src.reshape(-1)
    npf = num_pdf.reshape(-1)
    nw = np.exp(num_logw.reshape(-1)).astype(np.float32)
    ncore = nd % NCORES
    jj = nd // NCORES
    grp = ncore * S_NUM + nd
    order_n = np.argsort(grp, kind="stable")
    grp_s = grp[order_n]
    first = np.r_[True, grp_s[1:] != grp_s[:-1]]
    start_pos = np.where(first, np.arange(len(nd)), 0)
    cum = np.arange(len(nd)) - np.maximum.accumulate(start_pos)
    part_n = jj[order_n] * NSUB + (cum % NSUB)
    slot_n = cum // NSUB
    for c in range(NCORES):
        sel = ncore[order_n] == c
        K = int(slot_n[sel].max()) + 1 if sel.any() else 1
        Kmax[4] = max(Kmax[4], K)
        raw[(c, 4)] = sel

    for c in range(NCORES):
        for j in range(4):
            K = Kmax[j]
            sel = raw[(c, j)]
            ai = np.zeros((128, K), np.int32)
            xi = np.zeros((128, K), np.int32)
            wt = np.zeros((128, K), np.float32)   # den w: same for all utts
            p, k = part_s[sel], k_within[sel]
            ai[p, k] = e_src[sel]
            xi[p, k] = e_pdf[sel]
            wt[p, k] = e_w[sel]
            pc = per_core[c]
            pc["aidx"][j] = ai; pc["xidx"][j] = xi; pc["w"][j] = wt
        K = Kmax[4]
        sel = raw[(c, 4)]
        ai = np.zeros((128, K), np.int32)
        xi = np.zeros((128, K), np.int32)
        wt = np.zeros((128, K, B), np.float32)
        p, k = part_n[sel], slot_n[sel]
        ai[p, k] = rowof_num[ns[order_n][sel]]
        xi[p, k] = npf[order_n][sel]
        wt[p, k, uu[order_n][sel]] = nw[order_n][sel]
        pc = per_core[c]
        pc["aidx"][4] = ai; pc["xidx"][4] = xi; pc["w"][4] = wt

    G = np.zeros((128, 128), np.float32)
    for q in range(S_NUM // NCORES):
        for m in range(NSUB):
            G[q * NSUB + m, q] = 1.0

    A0 = np.zeros((NROWS, B), np.float32)
    A0[rowof_den, :] = np.exp(den_init).astype(np.float32)[:, None]
    for u in range(B):
        A0[rowof_num, u] = np.exp(num_init[u]).astype(np.float32)
    F = np.zeros((NROWS, B), np.float32)
    F[rowof_den, :] = np.exp(den_final).astype(np.float32)[:, None]
    for u in range(B):
        F[rowof_num, u] = np.exp(num_final[u]).astype(np.float32)

    return per_core, Kmax, G, A0, F


def _wrap_idx(flat):
    # dma_gather index layout: flat index i -> [i%16, i//16]; shipped as the
    # 16-partition base block, replicated to 128 partitions on device.
    w = flat.reshape(-1, 16).T
    return np.ascontiguousarray(w.astype(np.int16))


# ------------------------------------------------------------- bass program
# variant: timing diagnostics only (results are wrong for variant != 0).
#   1 = per-step AllGather replaced by a local HBM copy
#   2 = variant 1 + all per-step dma_gathers skipped
# xplan: (SHB, O_ch, Pch) live-lane shipping plan — per-core padded shard
#   bytes, per-chunk byte offset in the AllGathered blob, per-chunk live
#   utterance-lane prefix length.
def _build(Kmax, n_steps, variant=0, xplan=None):
    import concourse.bass as bass
    import concourse.tile as tile
    from concourse import bacc, mybir

    f32 = mybir.dt.float32
    KTOT = sum(Kmax)
    NIDX = 128 * KTOT
    offs = np.cumsum([0] + Kmax).tolist()

    nc = bacc.Bacc("TRN2", target_bir_lowering=False, debug=False,
                   num_devices=NCORES, num_swdge_queues=4)
    core_ids = list(range(NCORES))
    SP = (variant == 5)        # timing experiment: single-packet gathers
    do_cc = variant in (0, 2, 5)       # real per-step AllGather
    do_gather = variant in (0, 1, 5)   # real per-step dma_gathers
    qn = [0]                # round-robin SWDGE queue assignment for gathers

    def next_q():
        qn[0] = (qn[0] + 1) % 4
        return qn[0]

    u8 = mybir.dt.uint8
    KD = offs[4]            # total den K slots
    KN = Kmax[4]            # num-tile K slots
    XROW = XG * B // 2      # packed bytes per x row (2 int4 steps/byte)

    # x staged packed int4 (scale S4; hi nibble = steps 0-7 of the chunk, lo
    # nibble = steps 8-15), time-chunk-sharded (core c holds XG-step chunks
    # CPCX*c..); one device-side AllGather assembles the full table in HBM.
    # (Collectives cannot read IO tensors, so bounce through an internal one.)
    # Rows are utt-major: byte u*8+s holds step 16ch+s (hi nibble) /
    # 16ch+8+s (lo) of utt u. Only the live utt-lane prefix of each chunk
    # ships (per xplan); chunks are round-robin balanced across cores,
    # AllGathered as a blob, then copied into the gather table (dead lanes
    # stay garbage -- harmless, they are masked by the length blend).
    SHB, O_ch, Pch = xplan
    xt8s_in = nc.dram_tensor("xt8s", [SHB // 256, 256], u8, kind="ExternalInput").ap()
    xt8loc = nc.dram_tensor("xt8loc", [SHB // 256, 256], u8).ap()
    xtblob = nc.dram_tensor("xtblob", [NCORES * SHB // 256, 256], u8,
                            addr_space="Shared").ap()
    xt8 = nc.dram_tensor("xt8g", [NCHX * D, XROW], u8).ap()
    # all small side inputs packed into one fp16 tensor (fewer PJRT
    # transfers, half the bytes); converted to f32 on device once:
    # wden | wnum | gmat | fshard | init(packed, alpha cols only) | len
    f16 = mybir.dt.float16
    o_wd = 0
    o_wn = o_wd + KD           # num-tile utt ids [128,KN], then weights
    o_wv = o_wn + KN
    o_io = o_wv + KN           # iota row 0..B-1 (all partitions)
    o_gm = o_io + B
    o_fs = o_gm + 128          # 4 den cols (utt-constant) + B num cols
    o_ip = o_fs + 4 + B
    o_ln = o_ip + 4 + B
    SC = o_ln + 64
    side_in = nc.dram_tensor("side", [128, SC], f16, kind="ExternalInput").ap()
    NC16 = NIDX // 16
    sidx_in = nc.dram_tensor("sidx", [16, 2 * NC16], mybir.dt.int16, kind="ExternalInput").ap()
    out_t = nc.dram_tensor("out", [4, B], f32, kind="ExternalOutput").ap()

    shard64 = nc.dram_tensor("shard64", [SHARD, 64], f32).ap()
    TT = [nc.dram_tensor(f"table{i}", [NROWS, 64], f32, addr_space="Shared").ap()
          for i in range(2)]

    with tile.TileContext(nc) as tc:
        with tc.tile_pool(name="main", bufs=1) as pool, \
             tc.tile_pool(name="psum", bufs=1, space="PSUM") as psum:

            # assemble the x blob on device (D2D), then expand live-lane
            # chunk blocks into the fixed-stride gather table
            nc.scalar.dma_start(out=xt8loc[:], in_=xt8s_in[:])
            nc.gpsimd.collective_compute(
                "AllGather", mybir.AluOpType.bypass,
                replica_groups=[core_ids],
                ins=[xt8loc[:]], outs=[xtblob[:]])
            for ch in range(NCHX):
                P = Pch[ch]
                if P == 0:
                    continue
                nc.scalar.dma_start(
                    out=bass.AP(xt8.tensor, ch * D * XROW, [(XROW, D), (1, P * 8)]),
                    in_=bass.AP(xtblob.tensor, O_ch[ch], [(P * 8, D), (1, P * 8)]))

            # indices shipped as the 16-partition base block; replicate to 128
            aidx_t = pool.tile([128, NC16], mybir.dt.int16)
            xidx_t = pool.tile([128, NC16], mybir.dt.int16)
            for g in range(8):
                nc.sync.dma_start(out=aidx_t[16 * g:16 * (g + 1), :],
                                  in_=sidx_in[:, 0:NC16])
                nc.sync.dma_start(out=xidx_t[16 * g:16 * (g + 1), :],
                                  in_=sidx_in[:, NC16:2 * NC16])
            sstage = pool.tile([128, SC], f16)
            nc.sync.dma_start(out=sstage[:], in_=side_in[:])
            wden = pool.tile([128, KD], f32)
            nc.vector.tensor_copy(out=wden[:], in_=sstage[:, o_wd:o_wd + KD])
            # expand one-hot num weights: wnum[p,k,u] = wval*(u == wid)
            wid = pool.tile([128, KN], f32)
            nc.vector.tensor_copy(out=wid[:], in_=sstage[:, o_wn:o_wn + KN])
            wval = pool.tile([128, KN], f32)
            nc.vector.tensor_copy(out=wval[:], in_=sstage[:, o_wv:o_wv + KN])
            iota = pool.tile([128, B], f32)
            nc.vector.tensor_copy(out=iota[:], in_=sstage[:, o_io:o_io + B])
            wnum = pool.tile([128, KN, B], f32)
            nc.vector.tensor_tensor(
                out=wnum[:],
                in0=wid[:].unsqueeze(2).to_broadcast([128, KN, B]),
                in1=iota[:].unsqueeze(1).to_broadcast([128, KN, B]),
                op=mybir.AluOpType.is_equal)
            nc.vector.tensor_tensor(
                out=wnum[:], in0=wnum[:],
                in1=wval[:].unsqueeze(2).to_broadcast([128, KN, B]),
                op=mybir.AluOpType.mult)
            gmat = pool.tile([128, 128], f32)
            nc.vector.tensor_copy(out=gmat[:], in_=sstage[:, o_gm:o_gm + 128])
            fshard = pool.tile([128, 5, B], f32)
            nc.vector.tensor_copy(
                out=fshard[:, 0:4, :],
                in_=sstage[:, o_fs:o_fs + 4].unsqueeze(2).to_broadcast([128, 4, B]))
            nc.vector.tensor_copy(
                out=fshard[:, 4:5, :],
                in_=sstage[:, o_fs + 4:o_fs + 4 + B].unsqueeze(1))
            len64 = pool.tile([1, 64], f32)
            nc.vector.tensor_copy(out=len64[:], in_=sstage[0:1, o_ln:o_ln + 64])

            ones128 = pool.tile([128, 1], f32)
            nc.vector.memset(ones128[:], 1.0)
            ones1r = pool.tile([1, 128], f32)
            nc.vector.memset(ones1r[:], 1.0)
            logs64 = pool.tile([1, 64], f32)
            nc.vector.memset(logs64[:], 0.0)

            # shard ping-pong tiles ([p, tile, utt]); shard_t[t%2] = alpha_t
            shard_t = [pool.tile([128, 5, B], f32, name=f"shard{i}") for i in range(2)]
            iden = sstage[:, o_ip:o_ip + 4].unsqueeze(2).to_broadcast([128, 4, B])
            inum = sstage[:, o_ip + 4:o_ip + 4 + B].unsqueeze(1)
            nc.vector.tensor_copy(out=shard_t[0][:, 0:4, :], in_=iden)
            nc.vector.tensor_copy(out=shard_t[0][:, 4:5, :], in_=inum)
            # shard64 internal := initial shard, alpha in cols 0:B, zeros after
            s64init = pool.tile([128, 5, 64], f32)
            nc.vector.memset(s64init[:], 0.0)
            nc.vector.tensor_copy(out=s64init[:, 0:4, 0:B], in_=iden)
            nc.vector.tensor_copy(out=s64init[:, 4:5, 0:B], in_=inum)
            for jj in range(5):
                nc.scalar.dma_start(out=shard64[jj * 128:(jj + 1) * 128, :],
                                    in_=s64init[:, jj, :])

            ga_t = [pool.tile([128, Kmax[j], 64], f32, name=f"ga{j}")
                    for j in range(5)]
            gx_t = [pool.tile([128, Kmax[j], XCH * B], f32, name=f"gx{j}")
                    for j in range(5)]
            gxq_t = [pool.tile([128, Kmax[j], XROW], u8, name=f"gxq{j}")
                     for j in range(5)]
            if variant >= 2:
                # timing diagnostics: no gathers in-loop, so initialize once
                for j in range(5):
                    nc.vector.memset(ga_t[j][:], 0.5)
                    nc.vector.memset(gxq_t[j][:], 0x88)
            KMX = max(Kmax)
            nib_t = [pool.tile([128, KMX, XCH * B], u8, name=f"nib{i}")
                     for i in range(2)]
            bias_t = pool.tile([128, 1], f32)
            nc.vector.memset(bias_t[:], -8.0 * S4)
            srows = pool.tile([128, B], f32)
            numreg = pool.tile([25, 8 * B], f32)
            s64 = pool.tile([1, 64], f32)
            inv64 = pool.tile([1, 64], f32)
            ln64 = pool.tile([1, 64], f32)
            m64 = pool.tile([1, 64], f32)
            ccat = pool.tile([1, 128], f32)
            cb = pool.tile([128, 128], f32)
            tmp5 = pool.tile([128, 5, B], f32)
            tmp64 = pool.tile([1, 64], f32)

            for t in range(n_steps):
                T_dst = TT[t % 2]
                a_old = shard_t[t % 2]
                a_new = shard_t[(t + 1) % 2]
                rescale = (t % RS == RS - 1)

                # 1. exchange shards -> full table for this step
                if do_cc:
                    nc.gpsimd.collective_compute(
                        "AllGather", mybir.AluOpType.bypass,
                        replica_groups=[core_ids],
                        ins=[shard64[:]], outs=[T_dst[:]])
                else:
                    nc.scalar.dma_start(out=T_dst[0:SHARD, :], in_=shard64[:])

                # 2. gathers, split per grid tile (and per <=GCAP chunk)
                # so tile j's compute overlaps later tiles' gathers
                q = t % XCH
                chx = t // XG
                grp = (t % XG) // XCH       # 4-step group within the chunk
                for j in range(5):
                    base = offs[j] * 128
                    nj = Kmax[j] * 128
                    if t % XG == 0 and do_gather:
                        # packed int4 x rows for the next 16 steps
                        for o in range(0, nj, GCAP):
                            n = min(GCAP, nj - o)
                            go, gn = (base + o), n
                            nc.gpsimd.dma_gather(
                                gxq_t[j][:, o // 128:(o + n) // 128, :],
                                xt8[chx * D:(chx + 1) * D, :],
                                xidx_t[:, go // 16:(go + gn) // 16], n, n,
                                XROW, single_packet=SP, queue_num=next_q())
                    if q == 0:
                        # unpack nibble (hi: steps 0-7, lo: 8-15) from the
                        # utt-major rows into step-major [s*32+u] order, then
                        # E' = exp(S4*(v-8)) * w for 4 steps
                        nib = nib_t[j % 2][:, 0:Kmax[j], :]
                        nib4 = nib.rearrange("p k (s u) -> p k s u", s=XCH)
                        src4 = gxq_t[j][:] \
                            .rearrange("p k (u s) -> p k u s", u=B) \
                            [:, :, :, 4 * (grp % 2):4 * (grp % 2) + 4] \
                            .transpose([0, 1, 3, 2])
                        if grp < 2:
                            nc.vector.tensor_scalar(
                                out=nib4, in0=src4, scalar1=4, scalar2=None,
                                op0=mybir.AluOpType.logical_shift_right)
                        else:
                            nc.vector.tensor_scalar(
                                out=nib4, in0=src4, scalar1=15, scalar2=None,
                                op0=mybir.AluOpType.bitwise_and)
                        nc.scalar.activation(
                            out=gx_t[j][:], in_=nib,
                            func=mybir.ActivationFunctionType.Exp,
                            scale=S4, bias=bias_t[:])
                        if j < 4:
                            wb = wden[:, offs[j]:offs[j + 1]].unsqueeze(2) \
                                .unsqueeze(3).to_broadcast([128, Kmax[j], XCH, B])
                        else:
                            wb = wnum[:].unsqueeze(2) \
                                .to_broadcast([128, Kmax[j], XCH, B])
                        nc.vector.tensor_tensor(
                            out=gx_t[j][:].rearrange("p k (s b) -> p k s b", s=XCH),
                            in0=gx_t[j][:].rearrange("p k (s b) -> p k s b", s=XCH),
                            in1=wb, op=mybir.AluOpType.mult)
                    if do_gather:
                        for o in range(0, nj, GCAP):
                            n = min(GCAP, nj - o)
                            go, gn = (base + o), n
                            nc.gpsimd.dma_gather(
                                ga_t[j][:, o // 128:(o + n) // 128, :], T_dst[:],
                                aidx_t[:, go // 16:(go + gn) // 16], n, n, 64,
                                single_packet=SP, queue_num=next_q())

                # 3+4. per tile: z = a_src * (w*exp(x)), reduce over slots
                for j in range(5):
                    gav = ga_t[j][:, :, 0:B]
                    nc.vector.tensor_tensor(
                        out=gav, in0=gav,
                        in1=gx_t[j][:, :, q * B:(q + 1) * B],
                        op=mybir.AluOpType.mult)
                    nc.vector.tensor_reduce(
                        out=a_new[:, j, :],
                        in_=gav.transpose([0, 2, 1]),
                        axis=mybir.AxisListType.X,
                        op=mybir.AluOpType.add)

                # 5. num sub-row combine
                pnum = psum.tile([128, B], f32, space="PSUM")
                nc.tensor.matmul(out=pnum[:], lhsT=gmat[:], rhs=a_new[:, 4, :],
                                 start=True, stop=True)
                nc.vector.tensor_copy(out=a_new[:, 4, :], in_=pnum[:])

                # 6. masks + (periodic) scales
                nc.vector.tensor_scalar(
                    out=m64[:], in0=len64[:], scalar1=float(t), scalar2=None,
                    op0=mybir.AluOpType.is_gt)
                if rescale:
                    nc.scalar.dma_start(out=srows[:], in_=T_dst[0:128, 0:B])
                    nreg_view = bass.AP(T_dst.tensor, DEN_ROWS * 64,
                                        [(64, 25), (SHARD * 64, 8), (1, B)])
                    nc.scalar.dma_start(out=numreg[:], in_=nreg_view)
                    ps1 = psum.tile([1, B], f32, space="PSUM")
                    nc.tensor.matmul(out=ps1[:], lhsT=ones128[:], rhs=srows[:],
                                     start=True, stop=True)
                    nc.vector.tensor_copy(out=s64[0:1, 0:B], in_=ps1[:])
                    ps2 = psum.tile([1, 8 * B], f32, space="PSUM")
                    nc.tensor.matmul(out=ps2[:], lhsT=ones128[0:25, :],
                                     rhs=numreg[:], start=True, stop=True)
                    nc.vector.tensor_reduce(
                        out=s64[0:1, B:2 * B],
                        in_=ps2[:].rearrange("o (c b) -> o c b", c=8).transpose([0, 2, 1]),
                        axis=mybir.AxisListType.X, op=mybir.AluOpType.add)
                    nc.vector.tensor_scalar(
                        out=s64[:], in0=s64[:], scalar1=1e-30, scalar2=None,
                        op0=mybir.AluOpType.max)
                    nc.vector.reciprocal(out=inv64[:], in_=s64[:])
                    nc.scalar.activation(out=ln64[:], in_=s64[:],
                                         func=mybir.ActivationFunctionType.Ln)
                    nc.vector.tensor_tensor(out=tmp64[:], in0=m64[:], in1=ln64[:],
                                            op=mybir.AluOpType.mult)
                    nc.vector.tensor_tensor(out=logs64[:], in0=logs64[:],
                                            in1=tmp64[:], op=mybir.AluOpType.add)
                    nc.vector.tensor_tensor(out=ccat[0:1, 0:64], in0=m64[:],
                                            in1=inv64[:], op=mybir.AluOpType.mult)
                else:
                    nc.vector.tensor_copy(out=ccat[0:1, 0:64], in_=m64[:])
                # C2 = 1 - m  (both halves share m; write den/num halves)
                nc.vector.tensor_scalar(
                    out=tmp64[:], in0=m64[:], scalar1=-1.0, scalar2=1.0,
                    op0=mybir.AluOpType.mult, op1=mybir.AluOpType.add)
                nc.vector.tensor_copy(out=ccat[0:1, 64:128], in_=tmp64[:])

                # broadcast [1,128] -> [128,128]
                pbc = psum.tile([128, 128], f32, space="PSUM")
                nc.tensor.matmul(out=pbc[:], lhsT=ones1r[:],
                                 rhs=ccat[:], start=True, stop=True)
                nc.vector.tensor_copy(out=cb[:], in_=pbc[:])

                # 7. a_new = C1*a_new + C2*a_old
                c1_den = cb[:, 0:B].unsqueeze(1).to_broadcast([128, 4, B])
                c1_num = cb[:, B:2 * B].unsqueeze(1).to_broadcast([128, 1, B])
                c2_den = cb[:, 2 * B:3 * B].unsqueeze(1).to_broadcast([128, 4, B])
                c2_num = cb[:, 3 * B:4 * B].unsqueeze(1).to_broadcast([128, 1, B])
                nc.vector.tensor_tensor(out=a_new[:, 0:4, :], in0=a_new[:, 0:4, :],
                                        in1=c1_den, op=mybir.AluOpType.mult)
                nc.vector.tensor_tensor(out=a_new[:, 4:5, :], in0=a_new[:, 4:5, :],
                                        in1=c1_num, op=mybir.AluOpType.mult)
                nc.vector.tensor_tensor(out=tmp5[:, 0:4, :], in0=a_old[:, 0:4, :],
                                        in1=c2_den, op=mybir.AluOpType.mult)
                nc.vector.tensor_tensor(out=tmp5[:, 4:5, :], in0=a_old[:, 4:5, :],
                                        in1=c2_num, op=mybir.AluOpType.mult)
                nc.vector.tensor_tensor(out=a_new[:], in0=a_new[:], in1=tmp5[:],
                                        op=mybir.AluOpType.add)

                # 8. write shard for next exchange
                sh_view = bass.AP(shard64.tensor, 0, [(64, 128), (128 * 64, 5), (1, B)])
                nc.sync.dma_start(out=sh_view, in_=a_new[:])

            # ---- final partials ----
            a_fin = shard_t[n_steps % 2]
            nc.vector.tensor_tensor(out=a_fin[:], in0=a_fin[:], in1=fshard[:],
                                    op=mybir.AluOpType.mult)
            pd = psum.tile([1, 4 * B], f32, space="PSUM")
            nc.tensor.matmul(out=pd[:], lhsT=ones128[:],
                             rhs=a_fin[:, 0:4, :], start=True, stop=True)
            den_part = pool.tile([1, B], f32)
            nc.vector.tensor_reduce(
                out=den_part[:],
                in_=pd[:].rearrange("o (j b) -> o j b", j=4).transpose([0, 2, 1]),
                axis=mybir.AxisListType.X, op=mybir.AluOpType.add)
            pn = psum.tile([1, B], f32, space="PSUM")
            nc.tensor.matmul(out=pn[:], lhsT=ones128[:], rhs=a_fin[:, 4, :],
                             start=True, stop=True)
            num_part = pool.tile([1, B], f32)
            nc.vector.tensor_copy(out=num_part[:], in_=pn[:])

            nc.sync.dma_start(out=out_t[0:1, :], in_=den_part[:])
            nc.sync.dma_start(out=out_t[1:2, :], in_=num_part[:])
            nc.sync.dma_start(out=out_t[2:3, :], in_=logs64[0:1, 0:B])
            nc.sync.dma_start(out=out_t[3:4, :], in_=logs64[0:1, B:2 * B])

    nc.compile()
    return nc


_CACHE = {}


def _get_program(Kmax, n_steps, variant=0, xplan=None):
    key = (tuple(Kmax), n_steps, variant, xplan)
    if key not in _CACHE:
        _CACHE[key] = _build(Kmax, n_steps, variant, xplan)
    return _CACHE[key]


_EXEC_CACHE = {}


def _get_executor(nc):
    """Cached equivalent of bass2jax.run_bass_via_pjrt's inner jit call.

    run_bass_via_pjrt rebuilds jax.jit(shard_map(_body)) on every
    invocation, which re-traces and re-lowers (~1.4s/call of pure host
    overhead). Build the identical callable once per program and reuse it;
    the NEFF, transfers, and device execution are unchanged.
    """
    if id(nc) in _EXEC_CACHE:
        return _EXEC_CACHE[id(nc)]
    import jax
    from jax.sharding import Mesh, PartitionSpec
    from jax.experimental.shard_map import shard_map
    from concourse import mybir
    from concourse.bass2jax import (_bass_exec_p, install_neuronx_cc_hook,
                                    partition_id_tensor)

    install_neuronx_cc_hook()
    partition_name = (nc.partition_id_tensor.name
                      if nc.partition_id_tensor else None)
    in_names, out_names, out_avals, zero_outs = [], [], [], []
    for alloc in nc.m.functions[0].allocations:
        if not isinstance(alloc, mybir.MemoryLocationSet):
            continue
        name = alloc.memorylocations[0].name
        if alloc.kind == "ExternalInput":
            if name != partition_name:
                in_names.append(name)
        elif alloc.kind == "ExternalOutput":
            shape = tuple(alloc.tensor_shape)
            dtype = mybir.dt.np(alloc.dtype)
            out_avals.append(jax.core.ShapedArray(shape, dtype))
            zero_outs.append(np.zeros(shape, dtype))
            out_names.append(name)
    n_params = len(in_names)
    n_outs = len(out_avals)
    in_names_all = in_names + out_names
    if partition_name is not None:
        in_names_all.append(partition_name)

    def _body(*args):
        operands = list(args)
        if partition_name is not None:
            operands.append(partition_id_tensor())
        outs = _bass_exec_p.bind(
            *operands, out_avals=tuple(out_avals),
            in_names=tuple(in_names_all), out_names=tuple(out_names),
            lowering_input_output_aliases=(), sim_require_finite=True,
            sim_require_nnan=True, nc=nc)
        return tuple(outs)

    devices = jax.devices()[:NCORES]
    mesh = Mesh(np.asarray(devices), ("core",))
    in_specs = (PartitionSpec("core"),) * (n_params + n_outs)
    out_specs = (PartitionSpec("core"),) * n_outs
    donate = tuple(range(n_params, n_params + n_outs))
    sharded = jax.jit(shard_map(_body, mesh=mesh, in_specs=in_specs,
                                out_specs=out_specs, check_rep=False),
                      donate_argnums=donate, keep_unused=True)
    entry = (sharded, in_names, out_names, out_avals, zero_outs, mesh, devices)
    _EXEC_CACHE[id(nc)] = entry
    return entry


def _run_cached(nc, in_maps):
    """Execute via the cached jit callable; mirrors run_bass_via_pjrt.

    Inputs are staged with per-device device_puts issued from a thread pool
    (the axon tunnel gains ~25% from concurrent streams), then assembled
    into global sharded arrays without any host-side concatenation.
    """
    import jax
    from concurrent.futures import ThreadPoolExecutor
    from jax.sharding import NamedSharding, PartitionSpec

    sharded, in_names, out_names, out_avals, zero_outs, mesh, devices = \
        _get_executor(nc)
    sh = NamedSharding(mesh, PartitionSpec("core"))

    jobs = [(nm, c) for nm in in_names for c in range(NCORES)]

    def put(job):
        nm, c = job
        return jax.device_put(np.asarray(in_maps[c][nm]), devices[c])

    with ThreadPoolExecutor(8) as ex:
        futs = list(ex.map(put, jobs))
    dev_in = []
    for i, nm in enumerate(in_names):
        shards = futs[i * NCORES:(i + 1) * NCORES]
        shape0 = shards[0].shape
        gshape = (NCORES * shape0[0], *shape0[1:])
        dev_in.append(jax.make_array_from_single_device_arrays(
            gshape, sh, shards))
    concat_zeros = [np.zeros((NCORES * z.shape[0], *z.shape[1:]), z.dtype)
                    for z in zero_outs]
    out_arrs = sharded(*dev_in, *concat_zeros)
    return [
        {nm: np.asarray(out_arrs[i]).reshape(NCORES, *out_avals[i].shape)[c]
         for i, nm in enumerate(out_names)}
        for c in range(NCORES)]


LAST_EXEC_NS = None
LAST_RUN_S = None


def _prepare(x, x_lengths, den_src, den_dst, den_pdf, den_logw, den_init,
             den_final, num_src, num_dst, num_pdf, num_logw, num_init,
             num_final, n_steps=T, _variant=0):
    x = np.asarray(x, np.float32)
    x_lengths_np = np.asarray(x_lengths)
    args = [np.asarray(a) for a in (den_src, den_dst, den_pdf, den_logw,
                                    den_init, den_final, num_src, num_dst,
                                    num_pdf, num_logw, num_init, num_final)]
    per_core, Kmax, G, A0, F = _preprocess(*args, x_lengths_np)
    KTOT = sum(Kmax)

    # x -> int4 quantized (offset-8, scale S4), packed 2 steps/byte: row
    # (ch*D + p), byte s*B+u holds step 16ch+s in the hi nibble and step
    # 16ch+8+s in the lo nibble, for all utts. Staged time-chunk-sharded
    # (CPCX chunks per core, zero-padded to NCHX chunks); one device-side
    # AllGather assembles the full table.
    q4 = (np.clip(np.rint(x * (1.0 / S4)), -8, 7) + 8).astype(np.uint8)
    xt = np.zeros((NCHX * XG, D, B), np.uint8)
    xt[:T] = q4.transpose(1, 2, 0)               # [T, D, B]
    t16 = xt.reshape(NCHX, 2, XG // 2, D, B)     # [ch, half, s, D, B]
    xutt = np.ascontiguousarray(
        ((t16[:, 0] << 4) | t16[:, 1])           # [NCHX, s(8), D, B]
        .transpose(0, 2, 3, 1))                  # [NCHX, D, B, 8] utt-major
    # live-lane shipping plan: per chunk, only the utt-lane prefix that is
    # still inside some utterance's length ships; chunks are round-robin
    # assigned to cores to balance bytes, shards padded to a common size.
    lens = x_lengths_np.astype(np.int64)
    Pch, sizes = [], []
    for ch in range(NCHX):
        alive = np.nonzero(lens > XG * ch)[0]
        P = int(alive.max()) + 1 if alive.size else 0
        Pch.append(P)
        sizes.append(D * 8 * P)
    order = [[] for _ in range(NCORES)]
    loads = [0] * NCORES
    for ch in sorted(range(NCHX), key=lambda k: -sizes[k]):
        c = min(range(NCORES), key=lambda k: loads[k])
        order[c].append(ch)
        loads[c] += sizes[ch]
    core_bytes = loads
    SHB = 256 * ((max(core_bytes) + 255) // 256)
    O_ch = [0] * NCHX
    xt8_shards = []
    for c in range(NCORES):
        blob = np.zeros(SHB, np.uint8)
        off = 0
        for ch in order[c]:
            n = sizes[ch]
            O_ch[ch] = c * SHB + off
            if n:
                blob[off:off + n] = xutt[ch, :, :Pch[ch], :].reshape(-1)
            off += n
        xt8_shards.append(blob.reshape(SHB // 256, 256))
    xplan = (SHB, tuple(O_ch), tuple(Pch))

    len64 = np.zeros(64, np.float32)
    len64[0:B] = x_lengths_np.astype(np.float32)
    len64[B:2 * B] = x_lengths_np.astype(np.float32)

    # packed side-tensor layout (must match _build):
    KD = sum(Kmax[0:4])
    KN = Kmax[4]
    o_wd, o_wn = 0, KD
    o_wv = o_wn + KN
    o_io = o_wv + KN
    o_gm = o_io + B
    o_fs = o_gm + 128          # 4 den cols (utt-constant) + B num cols
    o_ip = o_fs + 4 + B
    o_ln = o_ip + 4 + B
    SC = o_ln + 64

    in_maps = []
    for c in range(NCORES):
        pc = per_core[c]
        aflat = np.concatenate([pc["aidx"][j].T.reshape(-1) for j in range(5)])
        xflat = np.concatenate([pc["xidx"][j].T.reshape(-1) for j in range(5)])
        # index order: i = (off_j + k)*128 + p  -> per tile k-major, partition
        # fastest; aidx[j].T is [K, 128] -> reshape(-1) gives exactly that.
        fsh = F[c * SHARD:(c + 1) * SHARD, :]     # [640, B]

        side = np.zeros((128, SC), np.float16)
        side[:, o_wd:o_wn] = np.concatenate([pc["w"][j] for j in range(4)], axis=1)
        w4 = pc["w"][4]                       # [128, KN, B] one-hot per slot
        side[:, o_wn:o_wv] = np.where(w4.any(axis=2), w4.argmax(axis=2), B + 7)
        side[:, o_wv:o_io] = w4.max(axis=2)
        side[:, o_io:o_gm] = np.arange(B, dtype=np.float16)[None, :]
        side[:, o_gm:o_fs] = G
        fsh5 = fsh.reshape(5, 128, B)
        a05 = A0[c * SHARD:(c + 1) * SHARD, :].reshape(5, 128, B)
        for j in range(4):
            side[:, o_fs + j] = fsh5[j, :, 0]
            side[:, o_ip + j] = a05[j, :, 0]
        side[:, o_fs + 4:o_fs + 4 + B] = fsh5[4]
        side[:, o_ip + 4:o_ip + 4 + B] = a05[4]
        side[0, o_ln:SC] = len64
        sidx = np.concatenate([_wrap_idx(aflat.astype(np.int16)),
                               _wrap_idx(xflat.astype(np.int16))], axis=1)
        in_maps.append({
            "xt8s": xt8_shards[c],
            "side": side,
            "sidx": np.ascontiguousarray(sidx),
        })

    nc = _get_program(Kmax, n_steps, _variant, xplan)
    return nc, in_maps, x_lengths_np


def kernel(x, x_lengths, den_src, den_dst, den_pdf, den_logw, den_init, den_final,
           num_src, num_dst, num_pdf, num_logw, num_init, num_final,
           n_steps=T, _want_results=False, _trace=False, _variant=0):
    global LAST_EXEC_NS, LAST_RUN_S
    import time as _time

    nc, in_maps, x_lengths_np = _prepare(
        x, x_lengths, den_src, den_dst, den_pdf, den_logw, den_init,
        den_final, num_src, num_dst, num_pdf, num_logw, num_init, num_final,
        n_steps=n_steps, _variant=_variant)
    _t0 = _time.time()
    try:
        results = _run_cached(nc, in_maps)
    except Exception:
        # fall back to the stock SPMD runner
        from concourse.bass_utils import run_bass_kernel_spmd
        res = run_bass_kernel_spmd(nc, in_maps, core_ids=list(range(NCORES)))
        results = res.results
    LAST_RUN_S = _time.time() - _t0
    outs = [results[c]["out"] for c in range(NCORES)]
    if _want_results:
        return outs, results

    den_tot = np.sum([o[0] for o in outs], axis=0)
    num_tot = np.sum([o[1] for o in outs], axis=0)
    logs_den = outs[0][2]
    logs_num = outs[0][3]
    den_ll = np.log(np.maximum(den_tot, 1e-300)) + logs_den
    num_ll = np.log(np.maximum(num_tot, 1e-300)) + logs_num
    objf = -(num_ll.sum() - den_ll.sum()) / x_lengths_np.sum()
    return np.float32(objf)

